# revision 1
# baseline (speedup 1.0000x reference)
# Trainium2 Bass kernel for nn_MultiHeadTransformer (B=2, S=2048, E=1024, H=16, FF=4096).
#
# Sharding: 8-way tensor/head parallel with ZERO collectives. The reference's
# "faithful raw view" reshape (b, s, 3E) -> (b, 3, H, s, Dh) means q/k/v of head h
# are contiguous 512KB slices of the flat qkv output buffer. Each core computes
# the qkv rows covering exactly the 6 flat blocks (q/k/v x 2 heads) it owns, does
# attention for its 2 heads, and because the inverse raw view maps head h's output
# to token rows [128h, 128(h+1)), the proj/LN/FFN are row-local to the core.
# Per-core offsets within the row-aligned scratch differ mod 3072; they are
# supplied as a tiny uint32 input and applied with one dynamic-offset DMA per
# slot, keeping a single SPMD program.
import numpy as np

B, S, E, H, DH, FF = 2, 2048, 1024, 16, 64, 4096
ROW = 3 * E            # 3072 qkv columns
BLK = S * DH           # 131072 elements per (type, head) block
NCORES = 8
P = 128
SCR = ROW + P * ROW    # per-(b,t) scratch: pad row + 128 rows
INV_SCALE = 1.0 / float(np.sqrt(E))

_cached = {}


def _build():
    import concourse.bacc as bacc
    import concourse.bass as bass
    import concourse.mybir as mybir
    import concourse.tile as tile
    from concourse.masks import make_identity

    f32 = mybir.dt.float32
    bf16 = mybir.dt.bfloat16   # attention/QKV path
    fp16 = mybir.dt.float16    # FFN path (finer mantissa for weight casts)
    f32r = mybir.dt.float32r
    u32 = mybir.dt.uint32
    AF = mybir.ActivationFunctionType
    ALU = mybir.AluOpType

    nc = bacc.Bacc(trn_type="TRN2", target_bir_lowering=False, debug=False,
                   num_devices=NCORES)

    xq = nc.dram_tensor("xq", [B, 3, 88, E], bf16, kind="ExternalInput").ap()
    xr = nc.dram_tensor("xr", [B, 2, P, E], f32, kind="ExternalInput").ap()
    wqkv = nc.dram_tensor("wqkv", [6, 8, P, 512], bf16, kind="ExternalInput").ap()
    bqkv = nc.dram_tensor("bqkv", [1, ROW], f32, kind="ExternalInput").ap()
    wproj = nc.dram_tensor("wproj", [2, 8, P, 512], f32, kind="ExternalInput").ap()
    bproj = nc.dram_tensor("bproj", [1, E], f32, kind="ExternalInput").ap()
    w1e = nc.dram_tensor("w1e", [8, 32, P, P], fp16, kind="ExternalInput").ap()
    b1e = nc.dram_tensor("b1e", [1, FF], f32, kind="ExternalInput").ap()
    w2 = nc.dram_tensor("w2", [2, 32, P, 512], fp16, kind="ExternalInput").ap()
    b2 = nc.dram_tensor("b2", [1, E], f32, kind="ExternalInput").ap()
    offs = nc.dram_tensor("offs", [1, 4], u32, kind="ExternalInput").ap()
    ones_in = nc.dram_tensor("ones", [P, 130], f32, kind="ExternalInput").ap()
    triu_in = nc.dram_tensor("triu", [P, P], bf16, kind="ExternalInput").ap()
    out = nc.dram_tensor("out", [B, 2, P, E], f32, kind="ExternalOutput").ap()

    mlist = [(b, hh) for b in range(B) for hh in range(2)]

    with tile.TileContext(nc) as tc:
        with tc.tile_pool(name="singles", bufs=1) as singles, \
             tc.tile_pool(name="dram", bufs=1, space="DRAM") as dram:

            ident = singles.tile([P, P], f32)
            make_identity(nc, ident)
            ones_r = singles.tile([1, P], f32r)
            nc.sync.dma_start(ones_r, ones_in[0:1, 0:P].bitcast(f32r))
            # triu[k, q] = 1 where q >= k (causal keep-mask in (k, q) layout)
            triu = singles.tile([P, P], bf16)
            nc.sync.dma_start(triu, triu_in)
            ident_bf = singles.tile([P, P], bf16)
            make_identity(nc, ident_bf)
            ident_h = singles.tile([P, P], fp16)
            make_identity(nc, ident_h)
            eps_t = singles.tile([P, 1], f32)
            nc.vector.memset(eps_t, 1e-5)
            bq_row = singles.tile([1, ROW], f32r)
            nc.sync.dma_start(bq_row, bqkv.bitcast(f32r))
            bp_row = singles.tile([1, E], f32r)
            nc.sync.dma_start(bp_row, bproj.bitcast(f32r))
            b1_row = singles.tile([1, FF], f32r)
            nc.sync.dma_start(b1_row, b1e.bitcast(f32r))
            b2_row = singles.tile([1, E], f32r)
            nc.sync.dma_start(b2_row, b2.bitcast(f32r))
            ones_col = singles.tile([P, 16], f32)
            nc.sync.dma_start(ones_col, ones_in[:, 0:16])
            ones_b = singles.tile([1, P], bf16)
            nc.vector.tensor_copy(ones_b, ones_r.bitcast(f32))
            bq_b = singles.tile([1, ROW], bf16)
            nc.vector.tensor_copy(bq_b, bq_row.bitcast(f32))
            b1_b = singles.tile([1, FF], fp16)
            nc.vector.tensor_copy(b1_b, b1_row.bitcast(f32))
            b2_b = singles.tile([1, E], fp16)
            nc.vector.tensor_copy(b2_b, b2_row.bitcast(f32))
            ones_h = singles.tile([1, P], fp16)
            nc.vector.tensor_copy(ones_h, ones_r.bitcast(f32))
            ones_h512 = singles.tile([1, 512], fp16)
            nc.vector.memset(ones_h512, 1.0)
            offs_sb = singles.tile([1, 4], u32)
            nc.sync.dma_start(offs_sb, offs)
            off_v = [nc.values_load(offs_sb[:, t:t + 1], min_val=0, max_val=ROW,
                                    skip_runtime_bounds_check=True)
                     for t in range(3)]

            SCR88 = ROW + 88 * ROW
            scr = [[dram.tile([SCR88], bf16, tag=f"scr{b}{t}",
                              name=f"scr{b}_{t}") for t in range(3)]
                   for b in range(B)]

            def transpose_into(pool, dst, src_ap, tag="tp", idt=None, dt_=f32):
                prows = src_ap.shape[0]
                pcols = src_ap.shape[1]
                idt = ident if idt is None else idt
                t_ps = pool.tile([P, P], dt_, tag=tag, name="t_ps")
                nc.tensor.transpose(t_ps[:pcols, :prows], src_ap,
                                    idt[:prows, :prows])
                nc.vector.tensor_copy(dst, t_ps[:pcols, :prows])

            # ---------------- Phase A: QKV ----------------
            slots = [(b, t) for b in range(B) for t in range(3)]
            with tc.tile_pool(name="qkv_ps", bufs=6, space="PSUM") as qkv_ps, \
                 tc.tile_pool(name="tpa", bufs=2, space="PSUM") as tpa, \
                 tc.tile_pool(name="qkv_sb", bufs=1) as qkv_sb, \
                 tc.tile_pool(name="wq_sb", bufs=6) as wq_sb:
                xT = qkv_sb.tile([P, 6, 8, 88], bf16)    # lhsT chunks per slot
                y_sb = qkv_sb.tile([88, 6, ROW], bf16)    # qkv rows per slot
                for m, (b, t) in enumerate(slots):
                    x_sb = qkv_sb.tile([88, E], bf16, tag="x_in", bufs=3,
                                       name=f"x_in{m}")
                    nc.sync.dma_start(x_sb, xq[b, t])
                    for kc in range(8):
                        transpose_into(tpa, xT[:, m, kc, :],
                                       x_sb[:, kc * P:(kc + 1) * P],
                                       idt=ident_bf, dt_=bf16)
                for grp in range(2):
                    ms = [3 * grp, 3 * grp + 1, 3 * grp + 2]
                    for n6 in range(6):
                        ns = slice(n6 * 512, (n6 + 1) * 512)
                        acc = [qkv_ps.tile([88, 512], f32, tag="acc",
                                           name=f"qa{grp}_{n6}_{i}")
                               for i in range(3)]
                        for i in range(3):
                            nc.tensor.matmul(acc[i], lhsT=ones_b[:, :88],
                                             rhs=bq_b[:, ns],
                                             start=True, stop=False)
                        for kc in range(8):
                            w_sb = wq_sb.tile([P, 512], bf16, name="wq")
                            nc.scalar.dma_start(w_sb, wqkv[n6, kc])
                            for i, m in enumerate(ms):
                                nc.tensor.matmul(acc[i], lhsT=xT[:, m, kc, :],
                                                 rhs=w_sb, start=False,
                                                 stop=(kc == 7))
                        for i, m in enumerate(ms):
                            nc.vector.tensor_copy(y_sb[:, m, ns], acc[i])
                    for m in ms:
                        b, t = slots[m]
                        for q4 in range(4):
                            dst = scr[b][t][bass.ds(off_v[t], 88 * ROW)]
                            nc.sync.dma_start(
                                dst.rearrange("(r c) -> r c", c=ROW)
                                [22 * q4:22 * (q4 + 1), :],
                                y_sb[22 * q4:22 * (q4 + 1), m, :])

            # -------- Phases B+C (outT spans B..C, ln spans C..D) --------
            ln_pool_cm = tc.tile_pool(name="ln_pool", bufs=4)
            ln_pool = ln_pool_cm.__enter__()
            outT_cm = tc.tile_pool(name="outT_sb", bufs=1)
            outT_pool = outT_cm.__enter__()
            outT_all = {}
            ln_all = {}
            hT = ln_pool.tile([P, 4, 8, P], f32r, tag="hT", bufs=1)

            # ---------------- Phase B: attention ----------------
            # q is processed in two 1024-wide halves so each outT accumulator
            # is 2 PSUM banks; bufs=2 lets two (head, half) pipelines overlap.
            with tc.tile_pool(name="at_ps", bufs=2, space="PSUM") as at_ps, \
                 tc.tile_pool(name="ot_ps", bufs=2, space="PSUM") as ot_ps, \
                 tc.tile_pool(name="at_sb", bufs=4) as at_sb, \
                 tc.tile_pool(name="qkv_in", bufs=3) as qkv_in, \
                 tc.tile_pool(name="head_sb", bufs=2) as head_sb:
                for mi, (b, hh) in enumerate(mlist):
                    base = ROW + hh * BLK
                    qT = head_sb.tile([64, S], bf16, tag="qT", name=f"qT{mi}")
                    kT = head_sb.tile([64, S], bf16, tag="kT", name=f"kT{mi}")
                    v_sb = head_sb.tile([P, 16, 65], bf16, tag="v",
                                        name=f"v{mi}")
                    nc.vector.tensor_copy(
                        v_sb[:, :, 64:65],
                        ones_col.rearrange("p (f o) -> p f o", o=1))
                    qn = qkv_in.tile([P, 16, DH], bf16, tag="qn",
                                     name=f"qn{mi}")
                    kn = qkv_in.tile([P, 16, DH], bf16, tag="kn",
                                     name=f"kn{mi}")
                    CH = BLK // 4
                    for c4 in range(4):
                        sl = slice(4 * c4, 4 * c4 + 4)
                        seg = slice(base + c4 * CH, base + (c4 + 1) * CH)
                        nc.sync.dma_start(
                            qn[:, sl, :], scr[b][0][seg]
                            .rearrange("(i p d) -> p i d", p=P, d=DH))
                        nc.sync.dma_start(
                            kn[:, sl, :], scr[b][1][seg]
                            .rearrange("(i p d) -> p i d", p=P, d=DH))
                        nc.sync.dma_start(
                            v_sb[:, sl, 0:64],
                            scr[b][2][seg]
                            .rearrange("(i p d) -> p i d", p=P, d=DH))
                    for i in range(16):
                        transpose_into(at_ps, qT[:, i * P:(i + 1) * P],
                                       qn[:, i, :], tag="sc", idt=ident_bf,
                                       dt_=bf16)
                        transpose_into(at_ps, kT[:, i * P:(i + 1) * P],
                                       kn[:, i, :], tag="sc", idt=ident_bf,
                                       dt_=bf16)
                    oT_sb = outT_pool.tile([64, S], f32, tag="oTsb",
                                           bufs=4, name=f"oTsb{mi}")
                    for hf in range(2):
                        Q0, Q1 = 1024 * hf, 1024 * hf + 1024
                        jmax = 8 if hf == 0 else 16
                        oT = ot_ps.tile([65, 1024], f32, tag="oT",
                                        name=f"oT{mi}_{hf}")
                        for j in range(jmax):
                            q0 = max(128 * j, Q0)
                            Qa = 512 * (q0 // 512)      # bank-aligned tile base
                            W = Q1 - Qa                  # tile span (<= 1024)
                            cuts = [q0] + [c for c in range(512 * (q0 // 512 + 1),
                                                            Q1 + 1, 512)]
                            sc_ps = at_ps.tile([P, 1024], f32, tag="sc",
                                               name=f"sc{mi}_{hf}_{j}")
                            a_sb = at_sb.tile([P, 1024], bf16, tag="a", bufs=6,
                                              name=f"a{mi}_{hf}_{j}")
                            for si in range(len(cuts) - 1):
                                qs, qe = cuts[si], cuts[si + 1]
                                nc.tensor.matmul(
                                    sc_ps[:, qs - Qa:qe - Qa],
                                    lhsT=kT[:, 128 * j:128 * j + P],
                                    rhs=qT[:, qs:qe], start=True, stop=True)
                            nc.scalar.activation(
                                a_sb[:, q0 - Qa:Q1 - Qa],
                                sc_ps[:, q0 - Qa:Q1 - Qa], AF.Exp,
                                scale=float(INV_SCALE))
                            if q0 == 128 * j:
                                nc.gpsimd.tensor_mul(
                                    a_sb[:, q0 - Qa:q0 - Qa + P],
                                    a_sb[:, q0 - Qa:q0 - Qa + P], triu)
                            for si in range(len(cuts) - 1):
                                qs, qe = cuts[si], cuts[si + 1]
                                gnb = qs // 512
                                nc.tensor.matmul(
                                    oT[:, qs - Q0:qe - Q0], lhsT=v_sb[:, j, :],
                                    rhs=a_sb[:, qs - Qa:qe - Qa],
                                    start=(j == 0),
                                    stop=(j == min(jmax - 1, 4 * gnb + 3)))
                        dnrow = at_sb.tile([1, 1024], f32, tag="dnrow", bufs=2,
                                           name=f"dnrow{mi}_{hf}")
                        nc.vector.tensor_copy(dnrow, oT[64:65, :])
                        dnd = dram.tile([2, 1024], f32, tag="dnd", bufs=2,
                                        name=f"dnd{mi}_{hf}")
                        nc.sync.dma_start(dnd[0:1, :], dnrow)
                        wrap = at_sb.tile([P, 8], f32, tag="wrap", bufs=2,
                                          name=f"wrap{mi}_{hf}")
                        nc.sync.dma_start(
                            wrap, dnd[0, :].rearrange("(p f) -> p f", f=8))
                        nc.vector.reciprocal(wrap, wrap)
                        nc.sync.dma_start(
                            dnd[1, :].rearrange("(p f) -> p f", f=8), wrap)
                        rrep = at_sb.tile([64, 1024], f32, tag="rrep", bufs=2,
                                          name=f"rrep{mi}_{hf}")
                        nc.sync.dma_start(
                            rrep, dnd[1:2, :].to_broadcast([64, 1024]))
                        nc.vector.tensor_mul(oT_sb[:, Q0:Q1], oT[0:64, :],
                                             rrep)
                    outT_all[(b, hh)] = oT_sb
                    oT_r = oT_sb.rearrange("d (t a) -> d a t", a=16)
                    for kc in range(8):
                        for ah in range(2):
                            nc.vector.tensor_copy(
                                hT[64 * ah:64 * ah + 64, mi, kc, :],
                                oT_r[:, 2 * kc + ah, :])

            # ---------------- Phase C: proj + residual + LN ----------------
            with tc.tile_pool(name="pj_ps", bufs=6, space="PSUM") as pj_ps, \
                 tc.tile_pool(name="pj_sb", bufs=2) as pj_sb, \
                 tc.tile_pool(name="wp_sb", bufs=6) as wp_sb:
                for mi, m in enumerate(mlist):
                    b, hh = m
                    r_sb = pj_sb.tile([P, E], f32, tag="r", name=f"r{mi}")
                    xr_sb = pj_sb.tile([P, E], f32, tag="xr", name=f"xr{mi}")
                    nc.sync.dma_start(xr_sb, xr[b, hh])
                    for ns_i in range(2):
                        ns = slice(ns_i * 512, (ns_i + 1) * 512)
                        acc = pj_ps.tile([P, 512], f32, tag="pacc",
                                         name=f"pa{mi}_{ns_i}")
                        nc.tensor.matmul(acc, lhsT=ones_r, rhs=bp_row[:, ns],
                                         start=True, stop=False)
                        for kc in range(8):
                            w_sb = wp_sb.tile([P, 512], f32r, name="wp")
                            nc.scalar.dma_start(
                                w_sb, wproj[ns_i, kc].bitcast(f32r))
                            nc.tensor.matmul(acc, lhsT=hT[:, mi, kc, :],
                                             rhs=w_sb, start=False,
                                             stop=(kc == 7))
                        nc.vector.tensor_add(r_sb[:, ns], acc, xr_sb[:, ns])
                    stats = pj_sb.tile([P, 2, 6], f32, tag="stats",
                                       name=f"st{mi}")
                    for sg in range(2):
                        nc.vector.bn_stats(stats[:, sg, :],
                                           r_sb[:, sg * 512:(sg + 1) * 512])
                    mv = pj_sb.tile([P, 2], f32, tag="mv", name=f"mv{mi}")
                    nc.vector.bn_aggr(mv, stats)
                    nc.scalar.activation(mv[:, 1:2], mv[:, 1:2], AF.Sqrt,
                                         bias=eps_t, scale=1.0)
                    nc.vector.reciprocal(mv[:, 1:2], mv[:, 1:2])
                    ln_m = ln_pool.tile([P, E], f32, tag="ln", name=f"ln{mi}")
                    nc.vector.tensor_scalar(
                        ln_m, r_sb, mv[:, 0:1], mv[:, 1:2],
                        ALU.subtract, ALU.mult)
                    ln_all[m] = ln_m

            outT_cm.__exit__(None, None, None)

            # ---------------- Phase D: FFN ----------------
            # h1T computed directly: lhsT = w1 block (e,f), rhs = lnT over all
            # four m-tiles (e, 4*128 tokens) -> h1T (f, tokens). No h1
            # transposes needed, and h1T slices feed w2 as lhsT directly.
            with tc.tile_pool(name="ff_ps", bufs=6, space="PSUM") as ff_ps, \
                 tc.tile_pool(name="tpd", bufs=2, space="PSUM") as tpd, \
                 tc.tile_pool(name="ff_sb", bufs=1) as ff_sb, \
                 tc.tile_pool(name="wf_sb", bufs=6) as wf_sb, \
                 tc.tile_pool(name="o_sb", bufs=2) as o_pool:
                lnT = ff_sb.tile([P, 8, 4, P], fp16)
                for mi, m in enumerate(mlist):
                    for kc in range(8):
                        transpose_into(tpd, lnT[:, kc, mi, :],
                                       ln_all[m][:, kc * P:(kc + 1) * P])
                h1T = ff_sb.tile([P, 32, 4, P], fp16)   # (f-part, fc, token)
                for fc in range(32):
                    acc = ff_ps.tile([P, 512], f32, tag="facc",
                                     name=f"fa{fc}")
                    nc.tensor.matmul(
                        acc, lhsT=b1_b[:, 128 * fc:128 * (fc + 1)],
                        rhs=ones_h512, start=True, stop=False)
                    for kc in range(8):
                        w_sb = wf_sb.tile([P, P], fp16, tag="w1s",
                                          name=f"w1_{fc}_{kc}")
                        nc.scalar.dma_start(w_sb, w1e[kc, fc])
                        nc.tensor.matmul(
                            acc, lhsT=w_sb,
                            rhs=lnT[:, kc, :, :],
                            start=False, stop=(kc == 7))
                    nc.scalar.activation(h1T[:, fc, :, :], acc, AF.Relu)
                o_acc = {}
                for ns_i in range(2):
                    ns = slice(ns_i * 512, (ns_i + 1) * 512)
                    acc = [ff_ps.tile([P, 512], f32, tag="facc",
                                      name=f"oa{ns_i}_{i}") for i in range(4)]
                    for mi in range(4):
                        nc.tensor.matmul(acc[mi], lhsT=ones_h,
                                         rhs=b2_b[:, ns],
                                         start=True, stop=False)
                    for kc in range(32):
                        w_sb = wf_sb.tile([P, 512], fp16, tag="w2s",
                                          name=f"w2_{ns_i}_{kc}")
                        nc.scalar.dma_start(w_sb, w2[ns_i, kc])
                        for mi in range(4):
                            nc.tensor.matmul(acc[mi], lhsT=h1T[:, kc, mi, :],
                                             rhs=w_sb, start=False,
                                             stop=(kc == 31))
                    o_acc[ns_i] = acc
                for mi, (b, hh) in enumerate(mlist):
                    o_sb = o_pool.tile([P, E], f32, tag="o", name=f"o{mi}")
                    for ns_i in range(2):
                        ns = slice(ns_i * 512, (ns_i + 1) * 512)
                        nc.vector.tensor_copy(o_sb[:, ns], o_acc[ns_i][mi])
                    nc.sync.dma_start(out[b, hh], o_sb)

            ln_pool_cm.__exit__(None, None, None)

    nc.compile()
    return nc


def _get_nc():
    if "nc" not in _cached:
        _cached["nc"] = _build()
    return _cached["nc"]


def _make_in_maps(inputs):
    x = np.ascontiguousarray(np.asarray(inputs["x"], dtype=np.float32))
    w_qkv = np.ascontiguousarray(np.asarray(inputs["w_qkv"], dtype=np.float32))
    b_qkv = np.asarray(inputs["b_qkv"], dtype=np.float32).reshape(1, ROW)
    w_proj = np.ascontiguousarray(np.asarray(inputs["w_proj"], dtype=np.float32))
    b_proj = np.asarray(inputs["b_proj"], dtype=np.float32).reshape(1, E)
    ln_g = np.asarray(inputs["ln_g"], dtype=np.float32)
    ln_b = np.asarray(inputs["ln_b"], dtype=np.float32)
    w1 = np.asarray(inputs["w1"], dtype=np.float32)
    b1 = np.asarray(inputs["b1"], dtype=np.float32)
    w2 = np.ascontiguousarray(np.asarray(inputs["w2"], dtype=np.float32))
    b2 = np.asarray(inputs["b2"], dtype=np.float32).reshape(1, E)

    w1e = (ln_g[:, None] * w1).astype(np.float32)
    b1e = (b1 + ln_b @ w1).reshape(1, FF).astype(np.float32)

    def tile_w(w, n_outer, n_k):
        # (K, N) -> (n_outer, n_k, 128, 512) with [no, kc] = w[kc*128:(kc+1)*128, no*512:(no+1)*512]
        K, N = w.shape
        assert K == n_k * P and N == n_outer * 512
        t = w.reshape(n_k, P, n_outer, 512).transpose(2, 0, 1, 3)
        return np.ascontiguousarray(t)

    import ml_dtypes
    w_qkv_t = tile_w(w_qkv, 6, 8).astype(ml_dtypes.bfloat16)
    w_proj_t = tile_w(w_proj, 2, 8)
    w1e_t = np.ascontiguousarray(
        w1e.reshape(8, P, 32, P).transpose(0, 2, 1, 3)).astype(np.float16)
    w2_t = tile_w(w2, 2, 32).astype(np.float16)

    ones_host = np.ones((P, 130), np.float32)
    triu_host = np.triu(np.ones((P, P))).astype(ml_dtypes.bfloat16)
    in_maps = []
    for c in range(NCORES):
        xq = np.zeros((B, 3, 88, E), ml_dtypes.bfloat16)
        offs = np.zeros((1, 4), np.uint32)
        for t in range(3):
            start = (16 * t + 2 * c) * BLK
            T0 = start // ROW
            offs[0, t] = ROW - (start - T0 * ROW)
            n = min(88, S - T0)
            for b in range(B):
                xq[b, t, :n] = x[b, T0:T0 + n]
        xr = np.zeros((B, 2, P, E), np.float32)
        for hh in range(2):
            h_ = 2 * c + hh
            for b in range(B):
                xr[b, hh] = x[b, P * h_:P * (h_ + 1)]
        in_maps.append({
            "xq": xq, "xr": xr, "offs": offs,
            "ones": ones_host, "triu": triu_host,
            "wqkv": w_qkv_t, "bqkv": b_qkv, "wproj": w_proj_t, "bproj": b_proj,
            "w1e": w1e_t, "b1e": b1e, "w2": w2_t, "b2": b2,
        })
    return in_maps


def _run(inputs, trace=False, trace_cores=None):
    import sys
    if "/opt/trn_rl_repo" not in sys.path:
        sys.path.insert(0, "/opt/trn_rl_repo")
    from concourse.bass_utils import run_bass_kernel_spmd
    nc = _get_nc()
    in_maps = _make_in_maps(inputs)
    kwargs = {}
    if trace:
        kwargs["trace"] = True
        if trace_cores is not None:
            kwargs["trace_cores"] = trace_cores
    res = run_bass_kernel_spmd(nc, in_maps, list(range(NCORES)), **kwargs)
    full = np.zeros((B, S, E), np.float32)
    for c in range(NCORES):
        o = res.results[c]["out"]
        for hh in range(2):
            h_ = 2 * c + hh
            for b in range(B):
                full[b, P * h_:P * (h_ + 1)] = o[b, hh]
    return full, res


def kernel(**inputs) -> np.ndarray:
    import sys
    if "/opt/trn_rl_repo" not in sys.path:
        sys.path.insert(0, "/opt/trn_rl_repo")
    full, _ = _run(inputs)
    return full



# revision 4
# speedup vs baseline: 1.1094x; 1.1094x over previous
# Trainium2 Bass kernel for nn_MultiHeadTransformer (B=2, S=2048, E=1024, H=16, FF=4096).
#
# Sharding: 8-way head/row parallel with zero collectives (same decomposition as
# the reference's "faithful raw view": core c computes qkv rows covering the six
# flat (type, head) blocks of its two heads, runs attention for those heads, and
# the inverse view makes proj/LN/FFN row-local).
#
# v2 schedule, engineered for PE continuity (HAM stays warm) and engine balance:
#  - host supplies x pre-transposed (xqT) and per-partition-contiguous weight
#    layouts; each weight is ONE big DMA issued on the sync queue.
#  - scalar (Act) engine does ONLY exp + relu + LN sqrt; all DMA triggers live
#    on sync (HWDGE) or gpsimd (SWDGE).
#  - q/k transposes done by XBAR DMA-transpose (2-byte, 14ns/16x128-tile),
#    not PE+DVE.
#  - attention per head, Act/PE lockstep j-pipeline; softmax denominator via
#    ones-row in v (M=65); normalization: DVE reciprocal + DRAM broadcast.
#  - QKV is slot-major (batch 0 first) so batch-0 attention (Act-bound)
#    overlaps batch-1 QKV (PE-bound).
#  - biases: b_proj folded into the residual input host-side, b1 via the relu's
#    per-partition bias (h1 is computed transposed), b_qkv/b2 via K=1 seed MMs.
import numpy as np

B, S, E, H, DH, FF = 2, 2048, 1024, 16, 64, 4096
ROW = 3 * E            # 3072 qkv columns
BLK = S * DH           # 131072 elements per (type, head) block
NCORES = 8
P = 128
INV_SCALE = 1.0 / float(np.sqrt(E))

_cached = {}


def _build():
    import concourse.bacc as bacc
    import concourse.bass as bass
    import concourse.mybir as mybir
    import concourse.tile as tile

    f32 = mybir.dt.float32
    bf16 = mybir.dt.bfloat16
    fp16 = mybir.dt.float16
    u32 = mybir.dt.uint32
    AF = mybir.ActivationFunctionType
    ALU = mybir.AluOpType

    nc = bacc.Bacc(trn_type="TRN2", target_bir_lowering=False, debug=False,
                   num_devices=NCORES)

    xqT_d = nc.dram_tensor("xqT", [P, B, 3, 8, 88], bf16,
                           kind="ExternalInput").ap()
    wq_d = nc.dram_tensor("wq", [P, 8, ROW], bf16, kind="ExternalInput").ap()
    bq_d = nc.dram_tensor("bq", [1, ROW], bf16, kind="ExternalInput").ap()
    wp_d = nc.dram_tensor("wp", [P, 8, E], bf16, kind="ExternalInput").ap()
    w1_d = nc.dram_tensor("w1", [P, 8, FF], fp16, kind="ExternalInput").ap()
    b1T_d = nc.dram_tensor("b1T", [P, 32], f32, kind="ExternalInput").ap()
    w2_d = nc.dram_tensor("w2", [P, 32, E], fp16, kind="ExternalInput").ap()
    b2_d = nc.dram_tensor("b2", [1, E], fp16, kind="ExternalInput").ap()
    xr_d = nc.dram_tensor("xr", [B, 2, P, E], f32, kind="ExternalInput").ap()
    offs_d = nc.dram_tensor("offs", [1, 4], u32, kind="ExternalInput").ap()
    triu_d = nc.dram_tensor("triu", [P, P], bf16, kind="ExternalInput").ap()
    ones_d = nc.dram_tensor("ones", [1, P], f32, kind="ExternalInput").ap()
    out_d = nc.dram_tensor("out", [B, 2, P, E], f32, kind="ExternalOutput").ap()

    slots = [(b, t) for b in range(B) for t in range(3)]

    with tile.TileContext(nc) as tc:
        with tc.tile_pool(name="singles", bufs=1) as singles, \
             tc.tile_pool(name="dram", bufs=1, space="DRAM") as dram:
            triu = singles.tile([P, P], bf16)
            nc.sync.dma_start(triu, triu_d)
            bq_row = singles.tile([1, ROW], bf16)
            nc.sync.dma_start(bq_row, bq_d)
            b2_row = singles.tile([1, E], fp16)
            nc.sync.dma_start(b2_row, b2_d)
            b1T_sb = singles.tile([P, 32], f32)
            nc.sync.dma_start(b1T_sb, b1T_d)
            ones_f = singles.tile([1, P], f32)
            nc.sync.dma_start(ones_f, ones_d)
            ones_b = singles.tile([1, P], bf16)
            nc.vector.tensor_copy(ones_b, ones_f)
            ones_h = singles.tile([1, P], fp16)
            nc.vector.tensor_copy(ones_h, ones_f)
            eps_t = singles.tile([P, 1], f32)
            nc.vector.memset(eps_t, 1e-5)
            offs_sb = singles.tile([1, 4], u32)
            nc.sync.dma_start(offs_sb, offs_d)
            off_v = [nc.values_load(offs_sb[:, t:t + 1], min_val=0,
                                    max_val=ROW,
                                    skip_runtime_bounds_check=True)
                     for t in range(3)]

            SCR88 = ROW + 88 * ROW
            scr = [[dram.tile([SCR88], bf16, tag=f"scr{b}{t}",
                              name=f"scr{b}_{t}") for t in range(3)]
                   for b in range(B)]

            # PSUM pool for phases A-C: acc(2) + sc(2) + oT(4) = 8 banks
            ps0_cm = tc.tile_pool(name="ps0", bufs=1, space="PSUM")
            ps0 = ps0_cm.__enter__()

            # ---------------- Phase A: QKV (slot-major, b=0 first) --------
            poolA_cm = tc.tile_pool(name="poolA", bufs=1)
            poolA = poolA_cm.__enter__()
            wq_sb = poolA.tile([P, 8, ROW], bf16)
            nc.sync.dma_start(wq_sb, wq_d)
            xqT_sb = poolA.tile([P, B, 3, 8, 88], bf16)
            nc.sync.dma_start(xqT_sb, xqT_d)

            for m, (b, t) in enumerate(slots):
                y = poolA.tile([88, ROW], bf16, tag="y", bufs=2,
                               name=f"y{m}")
                for n6 in range(6):
                    ns = slice(n6 * 512, (n6 + 1) * 512)
                    acc = ps0.tile([P, 512], f32, tag="acc", bufs=2,
                                   name=f"qa{m}_{n6}")
                    nc.tensor.matmul(acc[:88, :], lhsT=ones_b[:, :88],
                                     rhs=bq_row[:, ns], start=True,
                                     stop=False)
                    for kc in range(8):
                        nc.tensor.matmul(acc[:88, :],
                                         lhsT=xqT_sb[:, b, t, kc, :],
                                         rhs=wq_sb[:, kc, ns],
                                         start=False, stop=(kc == 7))
                    nc.vector.tensor_copy(y[:, ns], acc[:88, :])
                for q4 in range(4):
                    dst = scr[b][t][bass.ds(off_v[t], 88 * ROW)]
                    nc.sync.dma_start(
                        dst.rearrange("(r c) -> r c", c=ROW)
                        [22 * q4:22 * (q4 + 1), :],
                        y[22 * q4:22 * (q4 + 1), :])
            poolA_cm.__exit__(None, None, None)

            # w1/wp space reuses phase-A space; mid-pool holds cross-phase
            # tensors (hT for proj, lnT+h1T for FFN).
            w1pool_cm = tc.tile_pool(name="w1pool", bufs=1)
            w1pool = w1pool_cm.__enter__()
            midpool_cm = tc.tile_pool(name="midpool", bufs=1)
            midpool = midpool_cm.__enter__()
            hT = midpool.tile([P, 4, 8, P], bf16)
            lnT = midpool.tile([P, 8, 4, P], fp16)
            h1T = midpool.tile([P, 32, 4, P], fp16)
            atn_cm = tc.tile_pool(name="atn", bufs=1)
            atn = atn_cm.__enter__()

            # stage q/k/v for both batches (gpsimd SWDGE), then XBAR
            # transposes on sync.  qT_pair rows 0:64 = head A, 64:128 = B.
            qs_t, ks_t, vv, qT, kT = {}, {}, {}, {}, {}
            for b in range(B):
                qs_t[b] = atn.tile([P, 16, P], bf16, tag="qstage", bufs=1,
                                   name=f"qs{b}")
                ks_t[b] = atn.tile([P, 16, P], bf16, tag="kstage", bufs=1,
                                   name=f"ks{b}")
                for hh in range(2):
                    vv[(b, hh)] = atn.tile([P, 16, 65], bf16, tag="v",
                                           bufs=4, name=f"v{b}_{hh}")
                    nc.vector.memset(vv[(b, hh)][:, :, 64:65], 1.0)
                for c2 in range(2):
                    i8 = slice(8 * c2, 8 * c2 + 8)
                    for hh, cb in ((0, 0), (1, 64)):
                        base = ROW + hh * BLK + c2 * (BLK // 2)
                        seg = slice(base, base + BLK // 2)
                        nc.gpsimd.dma_start(
                            qs_t[b][:, i8, cb:cb + 64],
                            scr[b][0][seg].rearrange("(i p d) -> p i d",
                                                     p=P, d=DH))
                        nc.gpsimd.dma_start(
                            ks_t[b][:, i8, cb:cb + 64],
                            scr[b][1][seg].rearrange("(i p d) -> p i d",
                                                     p=P, d=DH))
                        nc.gpsimd.dma_start(
                            vv[(b, hh)][:, i8, 0:64],
                            scr[b][2][seg].rearrange("(i p d) -> p i d",
                                                     p=P, d=DH))
                qT[b] = atn.tile([P, S], bf16, tag="qT", bufs=2,
                                 name=f"qT{b}")
                kT[b] = atn.tile([P, S], bf16, tag="kT", bufs=2,
                                 name=f"kT{b}")
                for i in range(16):
                    nc.sync.dma_start(qT[b][:, P * i:P * (i + 1)],
                                      qs_t[b][:, i, :], transpose=True)
                    nc.sync.dma_start(kT[b][:, P * i:P * (i + 1)],
                                      ks_t[b][:, i, :], transpose=True)
                if b == 0:
                    # big FFN/proj weight loads: issued after the b0 XBARs
                    # so they don't head-of-line-block the sync queue.
                    w1_sb = w1pool.tile([P, 8, FF], fp16)
                    nc.sync.dma_start(w1_sb, w1_d)
                    wp_sb = w1pool.tile([P, 8, E], bf16)
                    nc.sync.dma_start(wp_sb, wp_d)

            # ---------------- Phase B+C: attention, proj, LN --------------
            for b in range(B):
                for hh in range(2):
                    mi = 2 * b + hh
                    hp = slice(64 * hh, 64 * hh + 64)
                    v_h = vv[(b, hh)]
                    oT_sb = atn.tile([64, S], bf16, tag="oTsb", bufs=2,
                                     name=f"oTsb{mi}")
                    for hf in range(2):
                        Q0 = 1024 * hf
                        jmax = 8 + 8 * hf
                        oT = ps0.tile([65, 1024], f32, tag="oT", bufs=2,
                                      name=f"oT{mi}_{hf}")
                        sc = ps0.tile([P, 1024], f32, tag="sc", bufs=1,
                                      name=f"sc{mi}_{hf}")
                        for j in range(jmax):
                            s = max(128 * j - Q0, 0)
                            cuts = ([s] if s >= 512 else [s, 512]) + [1024]
                            for ci in range(len(cuts) - 1):
                                cs, ce = cuts[ci], cuts[ci + 1]
                                nc.tensor.matmul(
                                    sc[:, cs:ce],
                                    lhsT=kT[b][hp, P * j:P * (j + 1)],
                                    rhs=qT[b][hp, Q0 + cs:Q0 + ce],
                                    start=True, stop=True)
                            a = atn.tile([P, 1024], bf16, tag="a", bufs=2,
                                         name=f"a{mi}_{hf}_{j}")
                            nc.scalar.activation(a[:, s:1024], sc[:, s:1024],
                                                 AF.Exp,
                                                 scale=float(INV_SCALE))
                            if 128 * j >= Q0:
                                nc.gpsimd.tensor_mul(a[:, s:s + P],
                                                     a[:, s:s + P], triu)
                            for ci in range(len(cuts) - 1):
                                cs, ce = cuts[ci], cuts[ci + 1]
                                stop_j = 4 * (ce // 512) + 8 * hf - 1
                                nc.tensor.matmul(
                                    oT[:, cs:ce], lhsT=v_h[:, j, :],
                                    rhs=a[:, cs:ce],
                                    start=(j == 0), stop=(j == stop_j))
                        rr = atn.tile([1, 1024], f32, tag="rrow", bufs=2,
                                      name=f"rr{mi}_{hf}")
                        nc.vector.reciprocal(rr, oT[64:65, :])
                        dnd = dram.tile([1, 1024], f32, tag="dnd", bufs=2,
                                        name=f"dnd{mi}_{hf}")
                        nc.gpsimd.dma_start(dnd, rr)
                        rrep = atn.tile([64, 1024], f32, tag="rrep", bufs=1,
                                        name=f"rrep{mi}_{hf}")
                        nc.gpsimd.dma_start(
                            rrep, dnd.to_broadcast([64, 1024]))
                        nc.vector.tensor_mul(oT_sb[:, Q0:Q0 + 1024],
                                             oT[0:64, :], rrep)
                    # scatter into proj-lhsT layout
                    oT_r = oT_sb.rearrange("d (t a) -> d a t", a=16)
                    for kc in range(8):
                        for ah in range(2):
                            nc.vector.tensor_copy(
                                hT[64 * ah:64 * ah + 64, mi, kc, :],
                                oT_r[:, 2 * kc + ah, :])
                    # proj + residual + LN for this m-tile
                    xr_sb = atn.tile([P, E], f32, tag="xr", bufs=1,
                                     name=f"xr{mi}")
                    nc.gpsimd.dma_start(xr_sb, xr_d[b, hh])
                    r_sb = atn.tile([P, E], f32, tag="r", bufs=1,
                                    name=f"r{mi}")
                    for ns_i in range(2):
                        ns = slice(ns_i * 512, (ns_i + 1) * 512)
                        pacc = ps0.tile([P, 512], f32, tag="acc", bufs=2,
                                        name=f"pa{mi}_{ns_i}")
                        for kc in range(8):
                            nc.tensor.matmul(pacc, lhsT=hT[:, mi, kc, :],
                                             rhs=wp_sb[:, kc, ns],
                                             start=(kc == 0),
                                             stop=(kc == 7))
                        nc.vector.tensor_add(r_sb[:, ns], pacc,
                                             xr_sb[:, ns])
                    stats = atn.tile([P, 2, 6], f32, tag="stats", bufs=2,
                                     name=f"st{mi}")
                    for sg in range(2):
                        nc.vector.bn_stats(stats[:, sg, :],
                                           r_sb[:, sg * 512:(sg + 1) * 512])
                    mv = atn.tile([P, 2], f32, tag="mv", bufs=2,
                                  name=f"mv{mi}")
                    nc.vector.bn_aggr(mv, stats)
                    nc.scalar.activation(mv[:, 1:2], mv[:, 1:2], AF.Sqrt,
                                         bias=eps_t, scale=1.0)
                    nc.vector.reciprocal(mv[:, 1:2], mv[:, 1:2])
                    ln_m = atn.tile([P, E], fp16, tag="ln", bufs=2,
                                    name=f"ln{mi}")
                    nc.vector.tensor_scalar(ln_m, r_sb, mv[:, 0:1],
                                            mv[:, 1:2], ALU.subtract,
                                            ALU.mult)
                    for kc in range(8):
                        nc.sync.dma_start(lnT[:, kc, mi, :],
                                          ln_m[:, P * kc:P * (kc + 1)],
                                          transpose=True)
            atn_cm.__exit__(None, None, None)
            ps0_cm.__exit__(None, None, None)

            # ---------------- Phase D: FFN --------------------------------
            w2pool_cm = tc.tile_pool(name="w2pool", bufs=1)
            w2pool = w2pool_cm.__enter__()

            psf_cm = tc.tile_pool(name="psf", bufs=1, space="PSUM")
            psf = psf_cm.__enter__()
            for fc in range(32):
                facc = psf.tile([P, 512], f32, tag="facc", bufs=4,
                                name=f"fa{fc}")
                for kc in range(8):
                    nc.tensor.matmul(facc,
                                     lhsT=w1_sb[:, kc, P * fc:P * (fc + 1)],
                                     rhs=lnT[:, kc, :, :],
                                     start=(kc == 0), stop=(kc == 7))
                nc.scalar.activation(h1T[:, fc, :, :], facc, AF.Relu,
                                     bias=b1T_sb[:, fc:fc + 1])
            psf_cm.__exit__(None, None, None)

            pso_cm = tc.tile_pool(name="pso", bufs=1, space="PSUM")
            pso = pso_cm.__enter__()
            oaccs = {}
            for ns_i in range(2):
                for mi in range(4):
                    oacc = pso.tile([P, 512], f32, tag="oacc", bufs=8,
                                    name=f"oa{ns_i}_{mi}")
                    ns = slice(ns_i * 512, (ns_i + 1) * 512)
                    nc.tensor.matmul(oacc, lhsT=ones_h,
                                     rhs=b2_row[:, ns], start=True,
                                     stop=False)
                    oaccs[(ns_i, mi)] = oacc
            for g in range(4):
                w2c = w2pool.tile([P, 8, E], fp16, tag="w2c", bufs=2,
                                  name=f"w2c{g}")
                nc.sync.dma_start(w2c, w2_d[:, 8 * g:8 * g + 8, :])
                for kk in range(8):
                    kcf = 8 * g + kk
                    for ns_i in range(2):
                        ns = slice(ns_i * 512, (ns_i + 1) * 512)
                        for mi in range(4):
                            nc.tensor.matmul(
                                oaccs[(ns_i, mi)],
                                lhsT=h1T[:, kcf, mi, :],
                                rhs=w2c[:, kk, ns],
                                start=False, stop=(kcf == 31))
            for mi in range(4):
                b, hh = mi // 2, mi % 2
                o_sb = w2pool.tile([P, E], f32, tag="o", bufs=2,
                                   name=f"o{mi}")
                for ns_i in range(2):
                    ns = slice(ns_i * 512, (ns_i + 1) * 512)
                    nc.vector.tensor_copy(o_sb[:, ns], oaccs[(ns_i, mi)])
                nc.gpsimd.dma_start(out_d[b, hh], o_sb)
            pso_cm.__exit__(None, None, None)
            w2pool_cm.__exit__(None, None, None)
            midpool_cm.__exit__(None, None, None)
            w1pool_cm.__exit__(None, None, None)

    nc.compile()
    return nc


def _get_nc():
    if "nc" not in _cached:
        _cached["nc"] = _build()
    return _cached["nc"]


def _make_in_maps(inputs):
    import ml_dtypes
    bf = ml_dtypes.bfloat16
    x = np.ascontiguousarray(np.asarray(inputs["x"], dtype=np.float32))
    w_qkv = np.asarray(inputs["w_qkv"], dtype=np.float32)
    b_qkv = np.asarray(inputs["b_qkv"], dtype=np.float32)
    w_proj = np.asarray(inputs["w_proj"], dtype=np.float32)
    b_proj = np.asarray(inputs["b_proj"], dtype=np.float32)
    ln_g = np.asarray(inputs["ln_g"], dtype=np.float32)
    ln_b = np.asarray(inputs["ln_b"], dtype=np.float32)
    w1 = np.asarray(inputs["w1"], dtype=np.float32)
    b1 = np.asarray(inputs["b1"], dtype=np.float32)
    w2 = np.asarray(inputs["w2"], dtype=np.float32)
    b2 = np.asarray(inputs["b2"], dtype=np.float32)

    w1e = ln_g[:, None] * w1                     # [E, FF]
    b1e = b1 + ln_b @ w1                         # [FF]

    wq_h = np.ascontiguousarray(
        w_qkv.reshape(8, P, ROW).transpose(1, 0, 2)).astype(bf)
    wp_h = np.ascontiguousarray(
        w_proj.reshape(8, P, E).transpose(1, 0, 2)).astype(bf)
    w1_h = np.ascontiguousarray(
        w1e.reshape(8, P, FF).transpose(1, 0, 2)).astype(np.float16)
    w2_h = np.ascontiguousarray(
        w2.reshape(32, P, E).transpose(1, 0, 2)).astype(np.float16)
    b1T_h = np.ascontiguousarray(b1e.reshape(32, P).T).astype(np.float32)
    bq_h = b_qkv.reshape(1, ROW).astype(bf)
    b2_h = b2.reshape(1, E).astype(np.float16)
    triu_h = np.triu(np.ones((P, P))).astype(bf)
    ones_h = np.ones((1, P), np.float32)

    in_maps = []
    for c in range(NCORES):
        xqT = np.zeros((P, B, 3, 8, 88), bf)
        offs = np.zeros((1, 4), np.uint32)
        for t in range(3):
            start = (16 * t + 2 * c) * BLK
            T0 = start // ROW
            offs[0, t] = ROW - (start - T0 * ROW)
            n = min(88, S - T0)
            for b in range(B):
                xs = x[b, T0:T0 + n]             # [n, E]
                xqT[:, b, t, :, :n] = np.ascontiguousarray(
                    xs.T).reshape(8, P, n).transpose(1, 0, 2)
        xr = np.zeros((B, 2, P, E), np.float32)
        for hh in range(2):
            h_ = 2 * c + hh
            for b in range(B):
                xr[b, hh] = x[b, P * h_:P * (h_ + 1)] + b_proj
        in_maps.append({
            "xqT": xqT, "xr": xr, "offs": offs,
            "ones": ones_h, "triu": triu_h,
            "wq": wq_h, "bq": bq_h, "wp": wp_h,
            "w1": w1_h, "b1T": b1T_h, "w2": w2_h, "b2": b2_h,
        })
    return in_maps


def _run(inputs, trace=False, trace_cores=None):
    import sys
    if "/opt/trn_rl_repo" not in sys.path:
        sys.path.insert(0, "/opt/trn_rl_repo")
    from concourse.bass_utils import run_bass_kernel_spmd
    nc = _get_nc()
    in_maps = _make_in_maps(inputs)
    kwargs = {}
    if trace:
        kwargs["trace"] = True
        if trace_cores is not None:
            kwargs["trace_cores"] = trace_cores
    res = run_bass_kernel_spmd(nc, in_maps, list(range(NCORES)), **kwargs)
    full = np.zeros((B, S, E), np.float32)
    for c in range(NCORES):
        o = res.results[c]["out"]
        for hh in range(2):
            h_ = 2 * c + hh
            for b in range(B):
                full[b, P * h_:P * (h_ + 1)] = o[b, hh]
    return full, res


def kernel(**inputs) -> np.ndarray:
    import sys
    if "/opt/trn_rl_repo" not in sys.path:
        sys.path.insert(0, "/opt/trn_rl_repo")
    full, _ = _run(inputs)
    return full
